# revision 47
# baseline (speedup 1.0000x reference)
"""Trainium2 Bass kernel for nn_Attn_52432960749709 (v5).

Computes, for E:[B,N,D], W1/W2:[D,D]:
    q = E @ W1 ; k = E @ W2
    scores = (q @ k^T) / sqrt(D)          # per batch, [N, N]
    out = softmax(scores, axis=1)         # normalize over rows n, per column m

Strategy (data parallel over B across 8 NeuronCores, one batch element per
core; the small DxD weights are folded on the host into M = W1 @ W2^T,
cast to fp16 along with E, and replicated):

    scores = E M E^T / sqrt(D); with G = E M:  scores[n,m] = sum_d' G[n,d'] E[m,d'],
    i.e. out^T[m,n] = (E M^T E^T)[m,n] -- computed per m-chunk with the
    projection HT = (M E^T)-chunk as the stationary side

    Per core:
      E^T     XBAR DMA-transpose loads straight from HBM (fp16, 16x128
              tiles, one [256,512] block per load) -- no PE/DVE transpose
              work at all.
      HT      per m-chunk [512, 128] projection via 16 small fp16 matmuls
              (this is G^T restricted to the chunk's m-columns; making it
              the stationary side means the only psum->SBUF copies are
              these small chunks, not the full [512, N] G^T).
      s^T     [128 m, 512 n] fp16 matmuls, rhs = ET directly; ACT
              exp(scale*s) -> fp16 strip with accum_out producing Z;
              DVE 1/Z then tensor_scalar normalize per chunk
      out     the normalized strip IS the row block of O^T: ships straight
              to HBM as [128, 2048] fp16 (4KB rows), no transpose-back.
              Host permutes axes and upcasts while unsharding.
      sched   phase A is paced by the 8 ET block loads: all 16 HT chunks
              are computed as their blocks land, with early s^T pieces of
              chunk 0 filling PE slack (chunk 0 finalizes+stores first in
              phase B); phase B streams chunks 1..14, with the last two
              chunks' exps split into 512-wide single-psum pieces; the
              whole last chunk runs at the very end as four 512 pieces so
              the only chain after the final matmul is one 512-wide exp,
              the Z-reduce, and two 1024-wide normalize+stores on separate
              HWDGE queues.

    Cost-model notes that shaped the schedule (TimelineSim):
      - every DMA completion sem costs +900ns before consumers can start;
        HWDGE descriptor gen (625ns) is a single shared device across
        queues, and all DMA transfers serialize on one DMA_ENGINES
        resource.
      - PSUM accumulation groups must be emitted as consecutive matmuls
        per region: interleaving open groups that share a bank, or
        re-opening a stopped group with start=False, corrupts results on
        hardware (verified empirically; the sim does not model it).
      - ACT is strictly in-order; a 1024-wide exp (1225ns incl accum read)
        injects ~370ns of lag over 512-piece exps (799ns vs 852ns of
        matmuls), which takes ~8 pieces to drain -- hence the piece train
        at the end.
"""

import math

import numpy as np

B, N, D = 8, 2048, 512
P = 128
DC = D // P  # 4 contraction chunks
NB = 512  # matmul moving free dim
MC = N // P  # 16 m-chunks per core
NBLK = 8  # ET load blocks (256 rows each)

_CACHE: dict = {}


def _patch_tile_drain():
    """This walrus build rejects >1 extra sem wait on one TPB_CTRL
    instruction, so split the end-of-kernel drain's wait set across chained
    SP NOPs (same engine, so program order preserves barrier semantics)."""
    import concourse.tile as tile
    from concourse.vector_clock import ScopedClock

    if getattr(tile.TileContext, "_drain_split_patched", False):
        return

    max_waits = 1

    def _drain_and_barrier_split(self, tick_clock, wait_clock):
        nc = self.nc
        drain_inst = nc.sync.drain()
        wait_clock.add_sem_waits(
            drain_inst.ins, ScopedClock({None: tick_clock.global_clock})
        )
        si = drain_inst.ins.sync_info
        waits = list(si.on_wait or []) if si is not None else []
        if len(waits) > max_waits:
            si.on_wait = waits[:max_waits]
            rest = waits[max_waits:]
            # spread the wait chain across engines so the end-of-kernel
            # drain isn't serialized through one SEQ; the all-engine
            # barrier below joins the per-engine chains
            engs = [nc.sync, nc.scalar, nc.vector, nc.gpsimd]
            ei = 0
            while rest:
                nop = engs[ei % len(engs)].nop(
                    nofuse=True, hint="drain_wait_split"
                )
                ei += 1
                chunk, rest = rest[:max_waits], rest[max_waits:]
                nsi = nop.ins.sync_info
                if nsi is None:
                    import bass_rust

                    nop.ins.sync_info = bass_rust.SyncInfo(
                        on_wait=chunk, on_update=[]
                    )
                else:
                    nsi.on_wait = chunk

        nc.all_engine_barrier()
        assert self.sems is not None
        popped = nc._tile_sem_poison_stack.pop()
        assert popped is self._sem_poison
        nc.clear_and_free_semaphores(list(self.sems.allocated().values()))
        nc.all_engine_barrier()

    tile.TileContext._drain_and_barrier = _drain_and_barrier_split
    tile.TileContext._drain_split_patched = True


def _split_multi_waits(nc):
    """This walrus build supports only one sem-wait command per instruction.
    Hoist extra waits onto same-engine NOPs inserted just before the
    instruction (engines execute in order, so semantics are preserved)."""
    import bass_rust
    import concourse.mybir as mybir

    ctr = 0
    for fn in nc.m.functions:
        for blk in fn.blocks:
            insts = blk.instructions
            out = []
            changed = False
            for inst in insts:
                si = inst.sync_info
                waits = list(si.on_wait) if (si is not None and si.on_wait) else []
                if len(waits) > 1:
                    changed = True
                    for w in waits[:-1]:
                        ctr += 1
                        nop = mybir.InstNoOp(name=f"I-waitsplit-{ctr}")
                        nop.engine = inst.engine
                        nop.sync_info = bass_rust.SyncInfo(
                            on_wait=[w], on_update=[]
                        )
                        nc.register_instruction(nop)
                        out.append(nop)
                    si.on_wait = waits[-1:]
                out.append(inst)
            if changed:
                blk.instructions = out


def _build_nc(repeat=1):
    import concourse.bass as bass
    import concourse.mybir as mybir
    import concourse.tile as tile

    _patch_tile_drain()

    dt = mybir.dt
    f32, f16 = dt.float32, dt.float16
    Exp = mybir.ActivationFunctionType.Exp
    X = mybir.AxisListType.X

    scale = 1.0 / math.sqrt(float(D))

    nc = bass.Bass()
    E_d = nc.dram_tensor("E", [N, D], f16, kind="ExternalInput")
    M_d = nc.dram_tensor("M", [D, D], f16, kind="ExternalInput")
    # Output is O^T (softmax result transposed): row block mi holds the
    # values for output columns m in [mi*128, (mi+1)*128).
    O_d = nc.dram_tensor("O", [N, N], f16, kind="ExternalOutput")

    with tile.TileContext(nc) as tc:
        with (
            tc.tile_pool(name="persist", bufs=1) as persist,
            tc.tile_pool(name="exps", bufs=8) as exps,
            tc.tile_pool(name="outs", bufs=6) as outs,
            tc.tile_pool(name="small", bufs=24) as small,
            tc.tile_pool(name="psum_s", bufs=3, space="PSUM") as psum_s,
            tc.tile_pool(name="psum_ht", bufs=1, space="PSUM") as psum_ht,
        ):
            ET = persist.tile([P, DC, N], f16, tag="ET")  # E^T  [d, n]
            HTs = persist.tile([P, MC, DC, P], f16, tag="HT")  # per-chunk G^T
            Msb = persist.tile([P, DC, D], f16, tag="M")  # M    [d, d']

            def emit_once():
                # ---- loads ----
                # XBAR transpose DMAs get fenced against neighboring regular
                # DMAs (a regular DMA makes the next transpose wait out its
                # completion sem +900ns), so the host ships M TRANSPOSED and
                # every load is a transpose DMA. M chunks interleave with ET
                # block 0 so the first projection slab (dc=0, needing only
                # M0+ET0) starts as early as the DMA pipeline allows.
                RB = N // NBLK  # 256 rows per ET block

                def load_m(c):
                    nc.sync.dma_start_transpose(
                        Msb[:, c, :], M_d[:, c * P : (c + 1) * P]
                    )

                def load_cols(c0, c1):
                    nc.sync.dma_start_transpose(
                        ET[:, :, c0:c1], E_d[c0:c1, :]
                    )

                load_m(0)
                load_cols(0, RB)
                for c in range(1, DC):
                    load_m(c)
                for j in range(1, NBLK):
                    load_cols(j * RB, (j + 1) * RB)

                def ht_chunk(mi):
                    # HT(mi): 16 small matmuls, then one psum->SBUF copy.
                    # NOTE: each dpc region's 4-matmul accumulation group
                    # must stay CONSECUTIVE -- interleaving open groups that
                    # share a PSUM bank (e.g. dc-outer order) corrupts
                    # results on hardware.
                    ps = psum_s.tile([P, 2 * NB], f32, tag="ps", name="ps_ht")
                    for dpc in range(DC):
                        for dc in range(DC):
                            nc.tensor.matmul(
                                ps[:, dpc * P : (dpc + 1) * P],
                                lhsT=Msb[:, dc, dpc * P : (dpc + 1) * P],
                                rhs=ET[:, dc, mi * P : (mi + 1) * P],
                                start=(dc == 0),
                                stop=(dc == DC - 1),
                            )
                    nc.vector.tensor_copy(
                        out=HTs[:, mi],
                        in_=ps[:, :D].rearrange("p (c m) -> p c m", c=DC),
                    )

                ssbs: dict = {}
                apart: dict = {}

                def get_parts(mi, nz):
                    if mi not in apart:
                        ssb = exps.tile([P, N], f16, tag="ssb", name="ssb")
                        zp = small.tile([P, 4], f32, tag="zp", name="zp")
                        apart[mi] = (ssb, zp)
                    return apart[mi]

                def s_half(mi, h, split_exp=False, z0=None):
                    # split_exp: one exp per 512-wide quarter (into zp slots
                    # z0, z0+1) instead of one 1024-wide exp into slot h --
                    # costs an extra SBUF access + accumulator read on ACT,
                    # but halves the final exp's latency, so it's used for
                    # the halves right before the endgame.
                    ssb, zp = get_parts(mi, 3)
                    ps = psum_s.tile([P, 2 * NB], f32, tag="ps", name="ps_a")
                    for q in range(2):
                        nb = 2 * h + q
                        for dpc in range(DC):
                            nc.tensor.matmul(
                                ps[:, q * NB : (q + 1) * NB],
                                lhsT=HTs[:, mi, dpc, :],
                                rhs=ET[:, dpc, nb * NB : (nb + 1) * NB],
                                start=(dpc == 0),
                                stop=(dpc == DC - 1),
                            )
                        if split_exp:
                            nc.scalar.activation(
                                ssb[:, nb * NB : (nb + 1) * NB],
                                ps[:, q * NB : (q + 1) * NB],
                                Exp,
                                scale=scale,
                                accum_out=zp[:, z0 + q : z0 + q + 1],
                            )
                    if not split_exp:
                        nc.scalar.activation(
                            ssb[:, h * 2 * NB : (h + 1) * 2 * NB],
                            ps,
                            Exp,
                            scale=scale,
                            accum_out=zp[:, h : h + 1],
                        )

                def fin(mi, nz=2):
                    ssb, zp = apart.pop(mi)
                    zs = small.tile([P, 1], f32, tag="zs")
                    nc.vector.reduce_sum(zs, zp[:, :nz], axis=X)
                    rv = small.tile([P, 1], f32, tag="rv")
                    nc.vector.reciprocal(rv, zs)
                    ssbs[mi] = (ssb, rv)

                def stage_b(mi):
                    ssb, rv = ssbs.pop(mi)
                    # dedicated buffers for the end chunks: no WAR wait on a
                    # recycled slot whose old store is still draining
                    sfx = f"_e{mi}" if mi >= MC - 3 else ""
                    osb = outs.tile([P, N], f16, tag="osb" + sfx)
                    if mi == MC - 2:
                        # split the second-to-last store so its transfers
                        # clear the (exclusive) DMA engine before the endgame
                        # chunk's stores need it; both on the sync queue -- a
                        # store on the scalar queue here would occupy the ACT
                        # SEQ (DMA issue + HWDGE gen) right when the endgame
                        # exps need it
                        for h2 in range(2):
                            seg = slice(h2 * N // 2, (h2 + 1) * N // 2)
                            nc.vector.tensor_scalar_mul(osb[:, seg], ssb[:, seg], rv)
                            nc.sync.dma_start(O_d[mi * P : (mi + 1) * P, seg], osb[:, seg])
                    else:
                        nc.vector.tensor_scalar_mul(osb[:], ssb[:], rv)
                        nc.sync.dma_start(O_d[mi * P : (mi + 1) * P, :], osb)

                # s piece for chunk mi (nw columns from col0), exp'd into
                # the chunk's ssb strip. acc=True sums it into a zp slot on
                # the ACT accumulator; acc=False skips the accumulator (and
                # its fixed 187ns read-out on the exp's critical path) and
                # reduces the fp16 strip into the zp slot on DVE instead --
                # used for the endgame pieces so the last exps stay short.
                def s_piece(mi, col0, nw, zi, acc=True):
                    ssb, zp = get_parts(mi, 4)
                    ps = psum_s.tile([P, 2 * NB], f32, tag="ps", name="ps_t")
                    for dpc in range(DC):
                        nc.tensor.matmul(
                            ps[:, :nw],
                            lhsT=HTs[:, mi, dpc, :],
                            rhs=ET[:, dpc, col0 : col0 + nw],
                            start=(dpc == 0),
                            stop=(dpc == DC - 1),
                        )
                    if acc:
                        nc.scalar.activation(
                            ssb[:, col0 : col0 + nw],
                            ps[:, :nw],
                            Exp,
                            scale=scale,
                            accum_out=zp[:, zi : zi + 1],
                        )
                    else:
                        nc.scalar.activation(
                            ssb[:, col0 : col0 + nw],
                            ps[:, :nw],
                            Exp,
                            scale=scale,
                        )
                        nc.vector.reduce_sum(
                            zp[:, zi : zi + 1], ssb[:, col0 : col0 + nw], axis=X
                        )

                # ---- phase A: HT chunks paced by ET column arrival, with
                # early h0 halves slotted in once blocks 0-3 are present.
                # Chunk 0's h1 also runs here in two 512-wide pieces so it
                # can finalize and store first thing in phase B, keeping the
                # DMA queue clear well before the endgame. ----
                fill_sched = {
                    3: lambda: s_half(0, 0),
                    4: lambda: s_half(1, 0),
                    5: lambda: s_half(2, 0),
                    6: lambda: s_piece(0, 2 * NB, NB, 1),
                    7: lambda: s_piece(0, 3 * NB, NB, 2),
                }
                def ht_chunk_early(mi):
                    # chunk 0 races M's staggered arrival: its dpc0/dpc1
                    # regions go dc-OUTER in a dedicated 2-bank psum tile
                    # (one open accumulation group per bank -- the only
                    # interleaving that is safe on HW), so those 8 matmuls
                    # run as M0/M1/M2/M3 land instead of all stalling on M3
                    ph = psum_ht.tile([P, 2, NB], f32, tag="ph", name="ps_hte")
                    for dc in range(DC):
                        for r in range(2):
                            nc.tensor.matmul(
                                ph[:, r, :P],
                                lhsT=Msb[:, dc, r * P : (r + 1) * P],
                                rhs=ET[:, dc, mi * P : (mi + 1) * P],
                                start=(dc == 0),
                                stop=(dc == DC - 1),
                            )
                    nc.vector.tensor_copy(out=HTs[:, mi, 0:2], in_=ph[:, :, :P])
                    ps = psum_s.tile([P, 2 * NB], f32, tag="ps", name="ps_ht")
                    for dpc in (2, 3):
                        for dc in range(DC):
                            nc.tensor.matmul(
                                ps[:, dpc * P : (dpc + 1) * P],
                                lhsT=Msb[:, dc, dpc * P : (dpc + 1) * P],
                                rhs=ET[:, dc, mi * P : (mi + 1) * P],
                                start=(dc == 0),
                                stop=(dc == DC - 1),
                            )
                    nc.vector.tensor_copy(
                        out=HTs[:, mi, 2:4],
                        in_=ps[:, 2 * P : 4 * P].rearrange("p (c m) -> p c m", c=2),
                    )

                prefilled = {0, 1, 2}
                for j in range(NBLK):
                    (ht_chunk_early if j == 0 else ht_chunk)(2 * j)
                    ht_chunk(2 * j + 1)
                    if j in fill_sched:
                        fill_sched[j]()

                # ---- phase B: chunk 0 finalizes immediately (its pieces all
                # ran in phase A), then the remaining chunks stream through;
                # the last chunk's h0 is prefilled early so only its h1
                # remains at the end. The second-to-last chunk splits its
                # exps per-512 so its Z closes (and its store clears the DMA
                # engine) well before the endgame chunk's stores arrive. ----
                fin(0, nz=3)
                stage_b(0)
                for mi in range(1, MC - 1):
                    if mi == MC - 2:
                        # the second-to-last chunk runs ENTIRELY as
                        # single-psum 512-wide pieces: ACT is strictly
                        # in-order and a 512-piece exp (799ns) only
                        # undercuts its matmuls (852ns) by ~53ns, so the
                        # piece train after the last 1024-wide exp must be
                        # ~8 long for that exp's 373ns of injected lag to
                        # drain before the endgame's final exp
                        for q in range(4):
                            s_piece(mi, q * NB, NB, q)
                        fin(mi, nz=4)
                    else:
                        if mi not in prefilled:
                            s_half(mi, 0)
                        if mi == MC - 3:
                            s_piece(mi, 2 * NB, NB, 1)
                            s_piece(mi, 3 * NB, NB, 2)
                            fin(mi, nz=3)
                        else:
                            s_half(mi, 1)
                            fin(mi)
                    stage_b(mi)

                # ---- endgame: the whole last chunk (four 512-wide pieces,
                # one exp each) runs at the very end: the 3.4us of matmuls
                # ahead of the final piece give ACT room to drain, so the
                # only chain exposed after the last matmul is one 512-wide
                # exp, the Z-reduce, and two 1024-wide normalize+store
                # pieces on separate HWDGE queues. ----
                mi = MC - 1
                for q in range(4):
                    s_piece(mi, q * NB, NB, q)
                ssb, zp = apart.pop(mi)
                zs = small.tile([P, 1], f32, tag="zs")
                nc.vector.reduce_sum(zs, zp, axis=X)
                rv = small.tile([P, 1], f32, tag="rv")
                nc.vector.reciprocal(rv, zs)
                osb = outs.tile([P, N], f16, tag="osb_end")
                for h2 in range(2):
                    seg = slice(h2 * N // 2, (h2 + 1) * N // 2)
                    nc.vector.tensor_scalar_mul(osb[:, seg], ssb[:, seg], rv)
                    eng = nc.sync if h2 == 0 else nc.scalar
                    eng.dma_start(O_d[mi * P : (mi + 1) * P, seg], osb[:, seg])

            for _rep in range(repeat):
                emit_once()

    _split_multi_waits(nc)
    return nc


def _get_core(repeat=1):
    """Build (once) the Bass module and its I/O metadata."""
    if ("core", repeat) in _CACHE:
        return _CACHE[("core", repeat)]

    import jax

    import concourse.mybir as mybir
    from concourse import bass2jax

    nc = _build_nc(repeat)
    bass2jax.install_neuronx_cc_hook()

    partition_name = (
        nc.partition_id_tensor.name if nc.partition_id_tensor else None
    )

    in_names = []
    out_names = []
    out_avals = []
    for alloc in nc.m.functions[0].allocations:
        if not isinstance(alloc, mybir.MemoryLocationSet):
            continue
        name = alloc.memorylocations[0].name
        if alloc.kind == "ExternalInput":
            if name != partition_name:
                in_names.append(name)
        elif alloc.kind == "ExternalOutput":
            out_names.append(name)
            out_avals.append(
                jax.core.ShapedArray(
                    tuple(alloc.tensor_shape), mybir.dt.np(alloc.dtype)
                )
            )
    in_names_all = list(in_names) + list(out_names)
    if partition_name is not None:
        in_names_all.append(partition_name)

    _CACHE[("core", repeat)] = (
        nc, partition_name, in_names, out_names, out_avals, in_names_all
    )
    return _CACHE[("core", repeat)]


def _bind_exec(nc, partition_name, in_names_all, out_names, out_avals, operands):
    from concourse import bass2jax

    if partition_name is not None:
        operands = operands + [bass2jax.partition_id_tensor()]
    return tuple(
        bass2jax._bass_exec_p.bind(
            *operands,
            out_avals=tuple(out_avals),
            in_names=tuple(in_names_all),
            out_names=tuple(out_names),
            lowering_input_output_aliases=(),
            sim_require_finite=True,
            sim_require_nnan=True,
            nc=nc,
        )
    )


def _shard_jit(body, n_in, n_out):
    import jax
    import numpy as _np
    from jax.sharding import Mesh, PartitionSpec
    from jax.experimental.shard_map import shard_map

    devices = jax.devices()[:B]
    mesh = Mesh(_np.asarray(devices), ("core",))
    in_specs = (PartitionSpec("core"),) * n_in
    out_specs = (PartitionSpec("core"),) * n_out
    return jax.jit(
        shard_map(
            body, mesh=mesh, in_specs=in_specs, out_specs=out_specs, check_rep=False
        ),
        keep_unused=True,
    )


def _get_runner(repeat=1):
    """Jitted SPMD runner: fn(*args) -> concatenated outputs."""
    if ("runner", repeat) in _CACHE:
        return _CACHE[("runner", repeat)]

    import jax
    import numpy as _np

    nc, partition_name, in_names, out_names, out_avals, in_names_all = _get_core(repeat)
    n_params = len(in_names)
    n_outs = len(out_avals)

    def _body(*args):
        return _bind_exec(
            nc, partition_name, in_names_all, out_names, out_avals, list(args)
        )

    fn = _shard_jit(_body, n_params + n_outs, n_outs)

    def pack(in_maps):
        concat_in = [
            _np.concatenate([_np.asarray(m[name]) for m in in_maps], axis=0)
            for name in in_names
        ]
        concat_zero = [
            _np.zeros((B * a.shape[0], *a.shape[1:]), a.dtype) for a in out_avals
        ]
        return [jax.device_put(a) for a in concat_in + concat_zero]

    _CACHE[("runner", repeat)] = (fn, pack, out_names, out_avals)
    return _CACHE[("runner", repeat)]


def _host_inputs(E, W1, W2):
    """Fold weights on host (f64 for accuracy, negligible 512^3 FLOPs) and
    cast activations/weights to the device fp16 layout."""
    E = np.ascontiguousarray(np.asarray(E), dtype=np.float32)
    W1 = np.asarray(W1, dtype=np.float32)
    W2 = np.asarray(W2, dtype=np.float32)
    # The device pipeline computes out[n,m] = softmax_col(E Mdev^T E^T)[m,n]
    # (Mdev = XBAR-transposed M input), and scores[n,m] = (E Mw^T E^T)[m,n],
    # so shipping Mw directly makes Mdev^T = Mw^T: exactly right.
    Mw = (W1.astype(np.float64) @ W2.astype(np.float64).T).astype(np.float16)
    E16 = E.astype(np.float16)
    return [{"E": E16[b], "M": Mw} for b in range(B)]


def kernel(E, W1, W2):
    fn, pack, out_names, out_avals = _get_runner()
    args = pack(_host_inputs(E, W1, W2))
    outs = fn(*args)
    o = np.asarray(outs[0])  # [8*N, N] fp16, per-batch O^T
    # device emits out^T per batch; permute back and upcast while unsharding
    return o.reshape(B, N, N).transpose(0, 2, 1).astype(np.float32)


if __name__ == "__main__":
    rng = np.random.default_rng(0)
    E = rng.standard_normal((B, N, D), dtype=np.float32)
    W1 = rng.standard_normal((D, D), dtype=np.float32) * (2.0 / (D + D)) ** 0.5
    W2 = rng.standard_normal((D, D), dtype=np.float32) * (2.0 / (D + D)) ** 0.5
    out = kernel(E=E, W1=W1, W2=W2)
    print(out.shape, out.dtype, out.sum())

